# revision 1
# baseline (speedup 1.0000x reference)
"""GroupedLinear Trainium2 kernel (8 NeuronCores, SPMD).

Computes y[b, g*256+o] = sum_i x[b, g*256+i] * W[g, o, i] + bias[g, o]
for x [8192, 4096] f32, W [16, 256, 256] f32, b [16, 256] f32.

Strategy
--------
Batch-sharded data parallel: core c owns x rows [1024c, 1024(c+1)) — no
communication (groups are independent, every core holds all of W).

Host prep puts every tensor in the exact layout the device consumes so the
kernel does zero on-chip transposes and every DMA line is >=4KB contiguous:
  xT_dev [32, 128, 1024]        [c, p, b] = x_core[b, 128c+p]
  WT_dev [128, 16, 2, 2, 128]   [i', g, k, oc, o'] = W[g, 128oc+o', 128k+i']
  bias   [128, 32]              [p, ot]  = b.flat[128*ot + p]
  yT_dev [2, 8, 128, 4, 512]    [tb, q, p, j, b'] = y_core[512tb+b', 512q+128j+p]

Device (per core): W stays SBUF-resident (32KB/partition); x streams through
a 3-deep ring of 2MB pieces; matmuls are W-stationary with x^T as the moving
operand (out = yT tile [o'=128 part, b=512 free], K=256 as two 128-chunks
accumulated in PSUM); float32r matmul dtype (1 cyc/row at N=512 — 4x the
fp32 rate, ~1e-4 rel err, far inside the resid_var<1e-4 gate family); bias
added during the PSUM->SBUF drain via per-partition tensor_scalar_add on DVE;
stores batched 1MB with DRAM layout matched to SBUF (8KB contiguous lines).
Loads issue on the Sync HWDGE ring, stores on Scalar's, so store issue never
queues behind a multi-MB load. Measured ~105 us/kernel at ~410 GB/s DMA.
"""

import numpy as np

import concourse.bacc as bacc
import concourse.mybir as mybir
import concourse.tile as tile
from concourse.bass_utils import run_bass_kernel_spmd

G = 16
B = 8192
F = 4096
NCORES = 8
BS = B // NCORES   # 1024 batch rows per core
NB = 2             # batch slabs per core
BT = BS // NB      # 512 = moving-operand width per matmul
NCH = 32           # contraction chunks of 128 (= F/128)
NQ = 8             # o-tile quads; quad q covers groups 2q, 2q+1
CPP = 4            # x chunks per ring piece (one piece per q)
Y_BATCH = 4        # o-tiles per output store (1MB)
MM_DT = mybir.dt.float32r

_NC_CACHE = None


def _build_nc():
    nc = bacc.Bacc("TRN2", target_bir_lowering=False, debug=False)
    xT = nc.declare_dram_parameter("xT", [NCH, 128, BS], MM_DT, isOutput=False)
    WT = nc.declare_dram_parameter("WT", [128, G, 2, 2, 128], MM_DT, isOutput=False)
    bias = nc.declare_dram_parameter("bias", [128, NQ * Y_BATCH],
                                     mybir.dt.float32, isOutput=False)
    yT = nc.declare_dram_parameter("yT", [NB, NQ, 128, Y_BATCH, BT],
                                   mybir.dt.float32, isOutput=True)

    with tile.TileContext(nc) as tc:
        with tc.tile_pool(name="wp", bufs=1) as wpool, \
             tc.tile_pool(name="xp", bufs=3) as xpool, \
             tc.tile_pool(name="yp", bufs=4) as ypool, \
             tc.tile_pool(name="ps", bufs=8, space="PSUM") as pspool:

            w_sb = wpool.tile([128, G * 2 * 2 * 128], MM_DT, tag="w")
            bias_sb = wpool.tile([128, NQ * Y_BATCH], mybir.dt.float32, tag="bias")

            WPW = 2 * 2 * 2 * 128   # w_sb cols per quad (2 groups)

            def load_w(q):
                nc.sync.dma_start(
                    out=w_sb[:, q * WPW:(q + 1) * WPW].rearrange(
                        "p (g k oc o) -> p g k oc o", g=2, k=2, oc=2),
                    in_=WT[:, 2 * q:2 * (q + 1)],
                )

            def load_x(q, x_sb):
                if q == 0:
                    # halve the first piece so the first matmul starts sooner
                    for h in range(2):
                        nc.sync.dma_start(
                            out=x_sb[:, h * 2 * BS:(h + 1) * 2 * BS].rearrange(
                                "p (c b) -> p c b", c=2),
                            in_=xT[h * 2:(h + 1) * 2].rearrange("c p b -> p c b"),
                        )
                else:
                    nc.sync.dma_start(
                        out=x_sb[:, :].rearrange("p (c b) -> p c b", c=CPP),
                        in_=xT[q * CPP:(q + 1) * CPP].rearrange("c p b -> p c b"),
                    )

            load_w(0)
            x_ring = {}
            x_ring[0] = xpool.tile([128, CPP * BS], MM_DT, tag="x", name="x0")
            load_x(0, x_ring[0])
            load_w(1)
            x_ring[1] = xpool.tile([128, CPP * BS], MM_DT, tag="x", name="x1")
            load_x(1, x_ring[1])
            for q in range(2, NQ):
                load_w(q)
            nc.sync.dma_start(out=bias_sb[:, :], in_=bias[:, :])

            for q in range(NQ):
                if q + 2 < NQ:
                    x_ring[q + 2] = xpool.tile([128, CPP * BS], MM_DT,
                                               tag="x", name=f"x{q + 2}")
                    load_x(q + 2, x_ring[q + 2])
                x_sb = x_ring[q]
                y_sbs = [ypool.tile([128, Y_BATCH * BT], mybir.dt.float32,
                                    tag=f"y{tb}", name=f"y{tb}_{q}")
                         for tb in range(NB)]
                for j in range(Y_BATCH):
                    ot = q * Y_BATCH + j
                    g, oc = divmod(ot, 2)
                    for tb in range(NB):
                        ps = pspool.tile([128, BT], mybir.dt.float32, tag="ps",
                                         name=f"ps{q}_{j}_{tb}")
                        for k in range(2):
                            c = 2 * g + k
                            widx = (g * 2 + k) * 2 + oc
                            nc.tensor.matmul(
                                ps[:, :],
                                lhsT=w_sb[:, widx * 128:(widx + 1) * 128],
                                rhs=x_sb[:, (c % CPP) * BS + tb * BT:
                                            (c % CPP) * BS + (tb + 1) * BT],
                                start=(k == 0), stop=(k == 1),
                            )
                        nc.vector.tensor_scalar_add(
                            y_sbs[tb][:, j * BT:(j + 1) * BT], ps[:, :],
                            bias_sb[:, ot:ot + 1],
                        )
                nhalf = 2 if q == NQ - 1 else 1
                for tb in range(NB):
                    for h in range(nhalf):
                        w0 = h * (Y_BATCH // nhalf)
                        w1 = (h + 1) * (Y_BATCH // nhalf)
                        nc.scalar.dma_start(
                            out=yT[tb, q, :, w0:w1, :],
                            in_=y_sbs[tb][:, w0 * BT:w1 * BT].rearrange(
                                "p (j b) -> p j b", j=w1 - w0),
                        )
    nc.compile()
    return nc


def _get_nc():
    global _NC_CACHE
    if _NC_CACHE is None:
        _NC_CACHE = _build_nc()
    return _NC_CACHE


def _prep_inputs(x, W, b):
    WT = np.ascontiguousarray(
        W.reshape(G, 2, 128, 2, 128).transpose(4, 0, 3, 1, 2))
    bias_dev = np.ascontiguousarray(b.reshape(F).reshape(NQ * Y_BATCH, 128).T)
    in_maps = []
    for c in range(NCORES):
        xc = np.ascontiguousarray(x[c * BS:(c + 1) * BS].T).reshape(NCH, 128, BS)
        in_maps.append({"xT": xc, "WT": WT, "bias": bias_dev})
    return in_maps


def _gather_output(results):
    outs = []
    for c in range(NCORES):
        yTc = results[c]["yT"]  # [2, 8, 128, 4, 512]
        outs.append(yTc.transpose(0, 4, 1, 3, 2).reshape(BS, F))
    return np.concatenate(outs, axis=0)


def run(x, W, b, trace=False, tmpdir=None):
    """Full pipeline; returns (y, BassKernelResults)."""
    x = np.ascontiguousarray(np.asarray(x, dtype=np.float32))
    W = np.ascontiguousarray(np.asarray(W, dtype=np.float32))
    b = np.ascontiguousarray(np.asarray(b, dtype=np.float32))
    nc = _get_nc()
    in_maps = _prep_inputs(x, W, b)
    res = run_bass_kernel_spmd(nc, in_maps, core_ids=list(range(NCORES)),
                               trace=trace, tmpdir=tmpdir)
    return _gather_output(res.results), res


def kernel(x, W, b):
    y, _ = run(x, W, b)
    return y



# revision 2
# speedup vs baseline: 1.8128x; 1.8128x over previous
"""GroupedLinear Trainium2 kernel (8 NeuronCores, SPMD).

Computes y[b, g*256+o] = sum_i x[b, g*256+i] * W[g, o, i] + bias[g, o]
for x [8192, 4096] f32, W [16, 256, 256] f32, b [16, 256] f32.

Strategy
--------
Group-sharded: core c owns groups {2c, 2c+1}, i.e. input columns
[512c, 512(c+1)) and the matching output columns, for the FULL batch.
No communication (groups are independent) and no W replication.

The kernel is HBM-bandwidth bound (~358 GB/s per core), so all device
traffic is bf16: host downcasts x and W (rel err ~4e-3 vs the 2e-2
gate), the device computes in bf16 matmuls with f32 PSUM accumulation,
and y is stored bf16 and upcast on the host. Per-core traffic drops to
8 MB x + 8 MB y + 0.25 MB W = 16.25 MB (vs 36 MB for the f32
batch-sharded version) -> ~46 us roofline.

Device layouts (host prepped so every DMA line is >=1 KB contiguous):
  xT   [4, 128, 8192]      bf16  [k, p, b]   = x[b, 512c + 128k + p]
  WT   [128, 4, 2, 128]    bf16  [i', j, k, o'] = W[2c + j//2,
                                     128*(j%2) + o', 128*(2*(j//2)+k) + i']
  bias [128, 4]            f32   [p, j]      = b[2c + j//2, 128*(j%2) + p]
  yT   [4, 128, 4, 4, 512] bf16  [ss, p, j, s, b'] = y[2048ss + 512s + b',
                                                       512c + 128j + p]

Per core: x streams in 16 half-MB pieces (4 batch super-slabs x 4
k-chunks) on the Sync HWDGE ring; W stays SBUF-resident. Matmuls are
W-stationary (4 consecutive N=512 matmuls per LDWEIGHTS), accumulating
K=256 as two 128-chunks into a [128, 2048] PSUM tile (4 banks; 2 such
tiles double-buffer). PSUM -> SBUF drains add bias and cast to bf16 in
2048-wide ops, alternating between DVE and ACT so neither engine
bottlenecks. Stores are 2 MB per super-slab on the Scalar HWDGE ring.
"""

import numpy as np
import ml_dtypes

import concourse.bacc as bacc
import concourse.mybir as mybir
import concourse.tile as tile
from concourse.bass_utils import run_bass_kernel_spmd

G = 16
B = 8192
F = 4096
NCORES = 8
GPC = 2            # groups per core
FPC = 512          # in/out features per core
KCH = 4            # 128-wide contraction chunks per core
NJ = 4             # 128-wide output tiles per core
NSS = 4            # batch super-slabs (stores)
SPS = 4            # slabs per super-slab
BT = 512           # matmul moving-operand width
SSB = SPS * BT     # 2048 batch rows per super-slab
MM_DT = mybir.dt.bfloat16
BF16 = ml_dtypes.bfloat16

_NC_CACHE = None


def _build_nc():
    nc = bacc.Bacc("TRN2", target_bir_lowering=False, debug=False)
    xT = nc.declare_dram_parameter("xT", [KCH, 128, B], MM_DT, isOutput=False)
    WT = nc.declare_dram_parameter("WT", [128, NJ, 2, 128], MM_DT, isOutput=False)
    bias = nc.declare_dram_parameter("bias", [128, NJ], mybir.dt.float32,
                                     isOutput=False)
    yT = nc.declare_dram_parameter("yT", [NSS, 128, NJ, SPS, BT], MM_DT,
                                   isOutput=True)

    with tile.TileContext(nc) as tc:
        with tc.tile_pool(name="wp", bufs=1) as wpool, \
             tc.tile_pool(name="xp", bufs=1) as xpool, \
             tc.tile_pool(name="yp", bufs=2) as ypool, \
             tc.tile_pool(name="ps", bufs=2, space="PSUM") as pspool:

            w_sb = wpool.tile([128, NJ * 2 * 128], MM_DT, tag="w")
            bias_sb = wpool.tile([128, NJ], mybir.dt.float32, tag="bias")
            x_sb = [xpool.tile([128, B], MM_DT, tag=f"x{k}", name=f"x{k}")
                    for k in range(KCH)]

            # W + bias on the scalar ring so x starts immediately on sync's
            nc.scalar.dma_start(
                out=w_sb.rearrange("p (j k o) -> p j k o", j=NJ, k=2),
                in_=WT[:, :],
            )
            nc.scalar.dma_start(out=bias_sb[:, :], in_=bias[:, :])

            # x pieces in super-slab-major order so ss0's matmuls gate on
            # only 1 MB of loads
            for ss in range(NSS):
                for k in range(KCH):
                    nc.sync.dma_start(
                        out=x_sb[k][:, ss * SSB:(ss + 1) * SSB],
                        in_=xT[k, :, ss * SSB:(ss + 1) * SSB],
                    )

            for ss in range(NSS):
                y_sb = ypool.tile([128, NJ * SPS * BT], MM_DT, tag="y",
                                  name=f"y{ss}")
                for j in range(NJ):
                    ps = pspool.tile([128, SPS * BT], mybir.dt.float32,
                                     tag="ps", name=f"ps{ss}_{j}")
                    for k in range(2):
                        kg = 2 * (j // 2) + k
                        wc = (j * 2 + k) * 128
                        for s in range(SPS):
                            t = ss * SPS + s
                            nc.tensor.matmul(
                                ps[:, s * BT:(s + 1) * BT],
                                lhsT=w_sb[:, wc:wc + 128],
                                rhs=x_sb[kg][:, t * BT:(t + 1) * BT],
                                start=(k == 0), stop=(k == 1),
                            )
                    eng = nc.vector if j % 2 == 0 else nc.scalar
                    if eng is nc.vector:
                        eng.tensor_scalar_add(
                            y_sb[:, j * SPS * BT:(j + 1) * SPS * BT],
                            ps[:, :], bias_sb[:, j:j + 1],
                        )
                    else:
                        eng.activation(
                            y_sb[:, j * SPS * BT:(j + 1) * SPS * BT],
                            ps[:, :], mybir.ActivationFunctionType.Identity,
                            bias=bias_sb[:, j:j + 1],
                        )
                nc.scalar.dma_start(
                    out=yT[ss],
                    in_=y_sb.rearrange("p (j s b) -> p j s b", j=NJ, s=SPS),
                )
    nc.compile()
    return nc


def _get_nc():
    global _NC_CACHE
    if _NC_CACHE is None:
        _NC_CACHE = _build_nc()
    return _NC_CACHE


def _prep_inputs(x, W, b):
    in_maps = []
    for c in range(NCORES):
        xc = x[:, c * FPC:(c + 1) * FPC].astype(BF16)
        xTc = np.ascontiguousarray(xc.T).reshape(KCH, 128, B)
        Wc = W[2 * c:2 * c + 2].reshape(GPC, 2, 128, 2, 128)  # gl,oc,o',k,i'
        WTc = np.ascontiguousarray(
            Wc.transpose(4, 0, 1, 3, 2).reshape(128, NJ, 2, 128).astype(BF16))
        bc = np.ascontiguousarray(
            b[2 * c:2 * c + 2].reshape(NJ, 128).T.astype(np.float32))
        in_maps.append({"xT": xTc, "WT": WTc, "bias": bc})
    return in_maps


def _gather_output(results):
    y = np.empty((B, F), dtype=np.float32)
    for c in range(NCORES):
        yTc = results[c]["yT"]  # [NSS, 128, NJ, SPS, BT] bf16
        y[:, c * FPC:(c + 1) * FPC] = (
            yTc.transpose(0, 3, 4, 2, 1).reshape(B, FPC).astype(np.float32))
    return y


def run(x, W, b, trace=False, tmpdir=None):
    """Full pipeline; returns (y, BassKernelResults)."""
    x = np.ascontiguousarray(np.asarray(x, dtype=np.float32))
    W = np.ascontiguousarray(np.asarray(W, dtype=np.float32))
    b = np.ascontiguousarray(np.asarray(b, dtype=np.float32))
    nc = _get_nc()
    in_maps = _prep_inputs(x, W, b)
    res = run_bass_kernel_spmd(nc, in_maps, core_ids=list(range(NCORES)),
                               trace=trace, tmpdir=tmpdir)
    return _gather_output(res.results), res


def kernel(x, W, b):
    y, _ = run(x, W, b)
    return y


# revision 4
# speedup vs baseline: 2.1433x; 1.1823x over previous
"""GroupedLinear Trainium2 kernel (8 NeuronCores, SPMD).

Computes y[b, g*256+o] = sum_i x[b, g*256+i] * W[g, o, i] + bias[g, o]
for x [8192, 4096] f32, W [16, 256, 256] f32, b [16, 256] f32.

Strategy
--------
Group-sharded: core c owns groups {2c, 2c+1}, i.e. input columns
[512c, 512(c+1)) and the matching output columns, for the FULL batch.
No communication (groups are independent) and no W replication.

The kernel is HBM-bandwidth bound (~358 GB/s per core), so all device
traffic is bf16: host downcasts x and W (rel err ~4e-3 vs the 2e-2
gate), the device computes in bf16 matmuls with f32 PSUM accumulation,
and y is stored bf16 and upcast on the host. Per-core traffic drops to
8 MB x + 8 MB y + 0.25 MB W = 16.25 MB (vs 36 MB for the f32
batch-sharded version) -> ~46 us roofline.

Device layouts (host prepped so every DMA line is >=1 KB contiguous):
  xT   [4, 128, 8192]      bf16  [k, p, b]   = x[b, 512c + 128k + p]
  WT   [128, 4, 2, 128]    bf16  [i', j, k, o'] = W[2c + j//2,
                                     128*(j%2) + o', 128*(2*(j//2)+k) + i']
  bias [128, 4]            f32   [p, j]      = b[2c + j//2, 128*(j%2) + p]
  yT   [4, 128, 4, 4, 512] bf16  [ss, p, j, s, b'] = y[2048ss + 512s + b',
                                                       512c + 128j + p]

Per core: x streams in 16 half-MB pieces (4 batch super-slabs x 4
k-chunks) on the Sync HWDGE ring; W stays SBUF-resident. Matmuls are
W-stationary (4 consecutive N=512 matmuls per LDWEIGHTS), accumulating
K=256 as two 128-chunks into a [128, 2048] PSUM tile (4 banks; 2 such
tiles double-buffer). PSUM -> SBUF drains add bias and cast to bf16 in
2048-wide ops, alternating between DVE and ACT so neither engine
bottlenecks. Stores are 2 MB per super-slab on the Scalar HWDGE ring.
"""

import numpy as np
import ml_dtypes

import concourse.bacc as bacc
import concourse.mybir as mybir
import concourse.tile as tile
from concourse.bass_utils import run_bass_kernel_spmd

G = 16
B = 8192
F = 4096
NCORES = 8
GPC = 2            # groups per core
FPC = 512          # in/out features per core
KCH = 4            # 128-wide contraction chunks per core
NJ = 4             # 128-wide output tiles per core
NSS = 4            # batch super-slabs (stores)
SPS = 4            # slabs per super-slab
BT = 512           # matmul moving-operand width
SSB = SPS * BT     # 2048 batch rows per super-slab
MM_DT = mybir.dt.bfloat16
BF16 = ml_dtypes.bfloat16

_NC_CACHE = None


def _build_nc():
    nc = bacc.Bacc("TRN2", target_bir_lowering=False, debug=False)
    xT = nc.declare_dram_parameter("xT", [KCH, 128, B], MM_DT, isOutput=False)
    WT = nc.declare_dram_parameter("WT", [128, NJ, 2, 128], MM_DT, isOutput=False)
    bias = nc.declare_dram_parameter("bias", [128, NJ], mybir.dt.float32,
                                     isOutput=False)
    yT = nc.declare_dram_parameter("yT", [NSS, 128, NJ, SPS, BT], MM_DT,
                                   isOutput=True)

    with tile.TileContext(nc) as tc:
        with tc.tile_pool(name="wp", bufs=1) as wpool, \
             tc.tile_pool(name="xp", bufs=1) as xpool, \
             tc.tile_pool(name="yp", bufs=3) as ypool, \
             tc.tile_pool(name="ps", bufs=4, space="PSUM") as pspool:

            w_sb = wpool.tile([128, NJ * 2 * 128], MM_DT, tag="w")
            bias_sb = wpool.tile([128, NJ], mybir.dt.float32, tag="bias")
            x_sb = [xpool.tile([128, B], MM_DT, tag=f"x{k}", name=f"x{k}")
                    for k in range(KCH)]

            # W + bias on the scalar ring so x starts immediately on sync's
            nc.scalar.dma_start(
                out=w_sb.rearrange("p (j k o) -> p j k o", j=NJ, k=2),
                in_=WT[:, :],
            )
            nc.scalar.dma_start(out=bias_sb[:, :], in_=bias[:, :])

            # x pieces in super-slab-major order so ss0's matmuls gate on
            # only 1 MB of loads
            for ss in range(NSS):
                for k in range(KCH):
                    nc.sync.dma_start(
                        out=x_sb[k][:, ss * SSB:(ss + 1) * SSB],
                        in_=xT[k, :, ss * SSB:(ss + 1) * SSB],
                    )

            HB = 2 * BT  # half a super-slab's batch per PSUM tile
            for ss in range(NSS):
                y_sb = ypool.tile([128, NJ * SPS * BT], MM_DT, tag="y",
                                  name=f"y{ss}")
                for j in range(NJ):
                    for h in range(2):
                        ps = pspool.tile([128, HB], mybir.dt.float32,
                                         tag="ps", name=f"ps{ss}_{j}_{h}")
                        for k in range(2):
                            kg = 2 * (j // 2) + k
                            wc = (j * 2 + k) * 128
                            for u in range(2):
                                t = ss * SPS + 2 * h + u
                                nc.tensor.matmul(
                                    ps[:, u * BT:(u + 1) * BT],
                                    lhsT=w_sb[:, wc:wc + 128],
                                    rhs=x_sb[kg][:, t * BT:(t + 1) * BT],
                                    start=(k == 0), stop=(k == 1),
                                )
                        yo = j * SPS * BT + h * HB
                        if h == 0:
                            nc.vector.tensor_scalar_add(
                                y_sb[:, yo:yo + HB], ps[:, :],
                                bias_sb[:, j:j + 1],
                            )
                        else:
                            nc.scalar.activation(
                                y_sb[:, yo:yo + HB], ps[:, :],
                                mybir.ActivationFunctionType.Identity,
                                bias=bias_sb[:, j:j + 1],
                            )
                    nc.scalar.dma_start(
                        out=yT[ss, :, j],
                        in_=y_sb[:, j * SPS * BT:(j + 1) * SPS * BT].rearrange(
                            "p (s b) -> p s b", s=SPS),
                    )
    nc.compile()
    return nc


def _get_nc():
    global _NC_CACHE
    if _NC_CACHE is None:
        _NC_CACHE = _build_nc()
    return _NC_CACHE


def _prep_inputs(x, W, b):
    in_maps = []
    for c in range(NCORES):
        xc = x[:, c * FPC:(c + 1) * FPC].astype(BF16)
        xTc = np.ascontiguousarray(xc.T).reshape(KCH, 128, B)
        Wc = W[2 * c:2 * c + 2].reshape(GPC, 2, 128, 2, 128)  # gl,oc,o',k,i'
        WTc = np.ascontiguousarray(
            Wc.transpose(4, 0, 1, 3, 2).reshape(128, NJ, 2, 128).astype(BF16))
        bc = np.ascontiguousarray(
            b[2 * c:2 * c + 2].reshape(NJ, 128).T.astype(np.float32))
        in_maps.append({"xT": xTc, "WT": WTc, "bias": bc})
    return in_maps


def _gather_output(results):
    y = np.empty((B, F), dtype=np.float32)
    for c in range(NCORES):
        yTc = results[c]["yT"]  # [NSS, 128, NJ, SPS, BT] bf16
        y[:, c * FPC:(c + 1) * FPC] = (
            yTc.transpose(0, 3, 4, 2, 1).reshape(B, FPC).astype(np.float32))
    return y


def run(x, W, b, trace=False, tmpdir=None):
    """Full pipeline; returns (y, BassKernelResults)."""
    x = np.ascontiguousarray(np.asarray(x, dtype=np.float32))
    W = np.ascontiguousarray(np.asarray(W, dtype=np.float32))
    b = np.ascontiguousarray(np.asarray(b, dtype=np.float32))
    nc = _get_nc()
    in_maps = _prep_inputs(x, W, b)
    res = run_bass_kernel_spmd(nc, in_maps, core_ids=list(range(NCORES)),
                               trace=trace, tmpdir=tmpdir)
    return _gather_output(res.results), res


def kernel(x, W, b):
    y, _ = run(x, W, b)
    return y


# revision 5
# speedup vs baseline: 2.2552x; 1.0522x over previous
"""GroupedLinear Trainium2 kernel (8 NeuronCores, SPMD).

Computes y[b, g*256+o] = sum_i x[b, g*256+i] * W[g, o, i] + bias[g, o]
for x [8192, 4096] f32, W [16, 256, 256] f32, b [16, 256] f32.

Strategy
--------
Group-sharded: core c owns groups {2c, 2c+1}, i.e. input columns
[512c, 512(c+1)) and the matching output columns, for the FULL batch.
No communication (groups are independent) and no W replication.

The kernel is HBM-bandwidth bound (~358 GB/s per core), so all device
traffic is bf16: host downcasts x and W (rel err ~4e-3 vs the 2e-2
gate), the device computes in bf16 matmuls with f32 PSUM accumulation,
and y is stored bf16 and upcast on the host. Per-core traffic drops to
8 MB x + 8 MB y + 0.25 MB W = 16.25 MB (vs 36 MB for the f32
batch-sharded version) -> ~46 us roofline.

Device layouts (host prepped so every DMA line is >=1 KB contiguous):
  xT   [4, 128, 8192]      bf16  [k, p, b]   = x[b, 512c + 128k + p]
  WT   [128, 4, 2, 128]    bf16  [i', j, k, o'] = W[2c + j//2,
                                     128*(j%2) + o', 128*(2*(j//2)+k) + i']
  bias [128, 4]            f32   [p, j]      = b[2c + j//2, 128*(j%2) + p]
  yT   [4, 128, 4, 4, 512] bf16  [ss, p, j, s, b'] = y[2048ss + 512s + b',
                                                       512c + 128j + p]

Per core: x streams in 16 half-MB pieces (4 batch super-slabs x 4
k-chunks) on the Sync HWDGE ring; W stays SBUF-resident. Matmuls are
W-stationary (4 consecutive N=512 matmuls per LDWEIGHTS), accumulating
K=256 as two 128-chunks into a [128, 2048] PSUM tile (4 banks; 2 such
tiles double-buffer). PSUM -> SBUF drains add bias and cast to bf16 in
2048-wide ops, alternating between DVE and ACT so neither engine
bottlenecks. Stores are 2 MB per super-slab on the Scalar HWDGE ring.
"""

import numpy as np
import ml_dtypes

import concourse.bacc as bacc
import concourse.mybir as mybir
import concourse.tile as tile
from concourse.bass_utils import run_bass_kernel_spmd

G = 16
B = 8192
F = 4096
NCORES = 8
GPC = 2            # groups per core
FPC = 512          # in/out features per core
KCH = 4            # 128-wide contraction chunks per core
NJ = 4             # 128-wide output tiles per core
NSS = 4            # batch super-slabs (stores)
SPS = 4            # slabs per super-slab
BT = 512           # matmul moving-operand width
SSB = SPS * BT     # 2048 batch rows per super-slab
MM_DT = mybir.dt.bfloat16
BF16 = ml_dtypes.bfloat16

_NC_CACHE = None


def _build_nc():
    nc = bacc.Bacc("TRN2", target_bir_lowering=False, debug=False)
    xT = nc.declare_dram_parameter("xT", [KCH, 128, B], MM_DT, isOutput=False)
    WT = nc.declare_dram_parameter("WT", [128, NJ, 2, 128], MM_DT, isOutput=False)
    bias = nc.declare_dram_parameter("bias", [128, NJ], mybir.dt.float32,
                                     isOutput=False)
    yT = nc.declare_dram_parameter("yT", [NSS, 128, NJ, SPS, BT], MM_DT,
                                   isOutput=True)

    with tile.TileContext(nc) as tc:
        with tc.tile_pool(name="wp", bufs=1) as wpool, \
             tc.tile_pool(name="xp", bufs=1) as xpool, \
             tc.tile_pool(name="yp", bufs=3) as ypool, \
             tc.tile_pool(name="ps", bufs=4, space="PSUM") as pspool:

            w_sb = wpool.tile([128, NJ * 2 * 128], MM_DT, tag="w")
            bias_sb = wpool.tile([128, NJ], mybir.dt.float32, tag="bias")
            x_sb = [xpool.tile([128, B], MM_DT, tag=f"x{k}", name=f"x{k}")
                    for k in range(KCH)]

            # W + bias on the scalar ring so x starts immediately on sync's
            nc.scalar.dma_start(
                out=w_sb.rearrange("p (j k o) -> p j k o", j=NJ, k=2),
                in_=WT[:, :],
            )
            nc.scalar.dma_start(out=bias_sb[:, :], in_=bias[:, :])

            # x pieces in super-slab-major order so ss0's matmuls gate on
            # only 1 MB of loads
            for ss in range(NSS):
                for k in range(KCH):
                    nc.sync.dma_start(
                        out=x_sb[k][:, ss * SSB:(ss + 1) * SSB],
                        in_=xT[k, :, ss * SSB:(ss + 1) * SSB],
                    )

            HB = 2 * BT  # half a super-slab's batch per PSUM tile
            for ss in range(NSS):
                y_sb = ypool.tile([128, NJ * SPS * BT], MM_DT, tag="y",
                                  name=f"y{ss}")
                for j in range(NJ):
                    for h in range(2):
                        ps = pspool.tile([128, HB], mybir.dt.float32,
                                         tag="ps", name=f"ps{ss}_{j}_{h}")
                        for k in range(2):
                            kg = 2 * (j // 2) + k
                            wc = (j * 2 + k) * 128
                            for u in range(2):
                                t = ss * SPS + 2 * h + u
                                nc.tensor.matmul(
                                    ps[:, u * BT:(u + 1) * BT],
                                    lhsT=w_sb[:, wc:wc + 128],
                                    rhs=x_sb[kg][:, t * BT:(t + 1) * BT],
                                    start=(k == 0), stop=(k == 1),
                                )
                        yo = j * SPS * BT + h * HB
                        if h == 0:
                            nc.vector.tensor_scalar_add(
                                y_sb[:, yo:yo + HB], ps[:, :],
                                bias_sb[:, j:j + 1],
                            )
                        else:
                            nc.scalar.activation(
                                y_sb[:, yo:yo + HB], ps[:, :],
                                mybir.ActivationFunctionType.Identity,
                                bias=bias_sb[:, j:j + 1],
                            )
                    nc.sync.dma_start(
                        out=yT[ss, :, j],
                        in_=y_sb[:, j * SPS * BT:(j + 1) * SPS * BT].rearrange(
                            "p (s b) -> p s b", s=SPS),
                    )
    nc.compile()
    return nc


def _get_nc():
    global _NC_CACHE
    if _NC_CACHE is None:
        _NC_CACHE = _build_nc()
    return _NC_CACHE


def _prep_inputs(x, W, b):
    in_maps = []
    for c in range(NCORES):
        xc = x[:, c * FPC:(c + 1) * FPC].astype(BF16)
        xTc = np.ascontiguousarray(xc.T).reshape(KCH, 128, B)
        Wc = W[2 * c:2 * c + 2].reshape(GPC, 2, 128, 2, 128)  # gl,oc,o',k,i'
        WTc = np.ascontiguousarray(
            Wc.transpose(4, 0, 1, 3, 2).reshape(128, NJ, 2, 128).astype(BF16))
        bc = np.ascontiguousarray(
            b[2 * c:2 * c + 2].reshape(NJ, 128).T.astype(np.float32))
        in_maps.append({"xT": xTc, "WT": WTc, "bias": bc})
    return in_maps


def _gather_output(results):
    y = np.empty((B, F), dtype=np.float32)
    for c in range(NCORES):
        yTc = results[c]["yT"]  # [NSS, 128, NJ, SPS, BT] bf16
        y[:, c * FPC:(c + 1) * FPC] = (
            yTc.transpose(0, 3, 4, 2, 1).reshape(B, FPC).astype(np.float32))
    return y


def run(x, W, b, trace=False, tmpdir=None):
    """Full pipeline; returns (y, BassKernelResults)."""
    x = np.ascontiguousarray(np.asarray(x, dtype=np.float32))
    W = np.ascontiguousarray(np.asarray(W, dtype=np.float32))
    b = np.ascontiguousarray(np.asarray(b, dtype=np.float32))
    nc = _get_nc()
    in_maps = _prep_inputs(x, W, b)
    res = run_bass_kernel_spmd(nc, in_maps, core_ids=list(range(NCORES)),
                               trace=trace, tmpdir=tmpdir)
    return _gather_output(res.results), res


def kernel(x, W, b):
    y, _ = run(x, W, b)
    return y
